# revision 1
# baseline (speedup 1.0000x reference)
"""Causal self-attention (RMS-normed QK + RoPE + v-mix) on 8 trn2 cores.

Sharding: tensor-parallel over heads x causal-balanced query split.
  - 12 heads -> 4 groups of 3 heads; group g runs on cores (2g, 2g+1).
  - Within a pair, core parity p owns the 8 query tiles with global tile
    index == p (mod 2) (128 rows each).
  - Each core emits a partial projection y_part for its 1024 query rows;
    the host sums the 4 group partials per row.

v2: all matmul operands bf16 (4x PE throughput vs fp32), k-side rms
scale folded into the exp activation's per-partition scale operand,
column-form sum-of-squares for K via tiny matmuls, rope combine done as
a PE identity-matmul accumulate, multiplicative post-exp masking,
reciprocal_approx_fast instead of the (3.8us/call) DVE reciprocal,
coalesced+ordered DMAs.
"""

import os
import sys

sys.path.insert(0, "/opt/trn_rl_repo")

import numpy as np

import concourse.bass as bass
from concourse import mybir
from concourse.tile import TileContext
from concourse.vector_clock import ScopedClock

F32 = mybir.dt.float32
BF16 = mybir.dt.bfloat16
AF = mybir.ActivationFunctionType

T = 2048
D = 768
NH = 12
HD = 64
HPC = 3  # heads per core
C = HPC * HD  # 192 channels per group
NQ = 1024  # query rows per core
NKT = T // 128  # 16 key tiles
NDT = D // 128  # 6 contraction tiles
EPS = float(np.finfo(np.float32).eps)

TRACE = False
TRACE_DIR = None
_CACHED = {}


def _patch_tile_tail():
    """walrus here rejects >1 sync-wait per instruction; TileContext's tail
    drain stacks one wait per active proc.  Spread them over wait_ge's."""
    if getattr(TileContext, "_tail_patched", False):
        return

    def _drain_and_barrier(self, tick_clock, wait_clock):
        nc = self.nc
        collector = nc.sync.nop()
        wait_clock.add_sem_waits(
            collector.ins, ScopedClock({None: tick_clock.global_clock})
        )
        si = collector.ins.sync_info
        waits = list(si.on_wait) if (si and si.on_wait) else []
        if len(waits) > 1:
            by_num = {h.num: h for h in wait_clock.sems.allocated().values()}
            kept, respawn = [], []
            for w in waits:
                if kept and w.id in by_num and w.wait_mode == "sem-ge-imm":
                    respawn.append(w)
                else:
                    kept.append(w)
            si.on_wait = kept
            for w in respawn:
                nc.sync.wait_ge(by_num[w.id], w.wait_value)
        nc.sync.drain()
        nc.all_engine_barrier()
        assert self.sems is not None
        popped = nc._tile_sem_poison_stack.pop()
        assert popped is self._sem_poison
        nc.clear_and_free_semaphores(list(self.sems.allocated().values()))
        nc.all_engine_barrier()

    TileContext._drain_and_barrier = _drain_and_barrier
    TileContext._tail_patched = True


def _split_multiwait_bir(bir_json):
    """Rewrite serialized BIR so no instruction carries more than one sync
    wait (this walrus build rejects >1): extra waits move onto single-wait
    NoOps inserted just before the instruction on the same engine."""
    import json as _json

    d = _json.loads(bir_json)
    for fn in d["functions"]:
        for bb in fn["blocks"]:
            out = []
            for inst in bb["instructions"]:
                si = inst.get("sync_info") or {}
                waits = si.get("on_wait") or []
                if len(waits) > 1:
                    for wi, w in enumerate(waits[:-1]):
                        out.append(
                            {
                                "name": f"{inst['name']}-wsplit{wi}",
                                "opcode": "EventSemaphore",
                                "engine": inst["engine"],
                                "debug": inst.get("debug", 0),
                                "ins": [],
                                "outs": [],
                                "sync_info": {"on_update": [], "on_wait": [w]},
                            }
                        )
                    si["on_wait"] = [waits[-1]]
                out.append(inst)
            bb["instructions"] = out
    enc = _json.dumps(d)
    return enc.encode() if isinstance(bir_json, bytes) else enc


def _patch_wait_split():
    import concourse.bass_utils as bu
    import concourse.bass2jax as b2j

    if getattr(bu, "_wait_split_patched", False):
        return
    orig = bu.compile_bir_kernel

    def wrapped(bir_json, tmpdir, neff_name="file.neff"):
        return orig(_split_multiwait_bir(bir_json), tmpdir, neff_name=neff_name)

    bu.compile_bir_kernel = wrapped
    b2j.compile_bir_kernel = wrapped
    bu._wait_split_patched = True


def j0_of(k):
    # first compact q-block (0..7) whose global tile can see key tile k,
    # under the uniform bound (odd-parity core's view; even cores get one
    # fully-masked diagonal block per odd k via the data mask)
    return k // 2


def build_nc():
    KPHASE = int(os.environ.get("KPHASE", "3"))
    SKIP = set(os.environ.get("KSKIP", "").split(","))
    _patch_tile_tail()
    _patch_wait_split()
    nc = bass.Bass("TRN2")

    # chunk-major packed inputs (see _host_prep for layouts)
    xtp = nc.dram_tensor("xtp", [128, 4 * NDT * 512], BF16, kind="ExternalInput")
    xqp = nc.dram_tensor("xqp", [128, 2 * NDT * 512], BF16, kind="ExternalInput")
    wq = nc.dram_tensor("wq", [128, NDT * C], BF16, kind="ExternalInput")
    wk = nc.dram_tensor("wk", [128, NDT * C], BF16, kind="ExternalInput")
    wv = nc.dram_tensor("wv", [128, NDT * C], BF16, kind="ExternalInput")
    wpp = nc.dram_tensor("wpp", [64, 3 * D], BF16, kind="ExternalInput")
    vin = nc.dram_tensor("vin", [128, NKT * C], F32, kind="ExternalInput")
    c4k = nc.dram_tensor("c4k", [128, T], BF16, kind="ExternalInput")
    s4k = nc.dram_tensor("s4k", [128, T], BF16, kind="ExternalInput")
    c4q = nc.dram_tensor("c4q", [128, NQ], BF16, kind="ExternalInput")
    s4q = nc.dram_tensor("s4q", [128, NQ], BF16, kind="ExternalInput")
    m01 = nc.dram_tensor("m01", [128, NKT * 128], BF16, kind="ExternalInput")
    perm = nc.dram_tensor("perm", [128, 128], BF16, kind="ExternalInput")
    idw = nc.dram_tensor("idw", [128, 128], BF16, kind="ExternalInput")
    qso = nc.dram_tensor("qso", [128, 131], BF16, kind="ExternalInput")
    yp = nc.dram_tensor("yp", [NQ, D], F32, kind="ExternalOutput")

    with TileContext(nc) as tc:
        with (
            tc.tile_pool(name="const", bufs=1) as constp,
            tc.tile_pool(name="persist", bufs=1) as pers,
            tc.tile_pool(name="vpool", bufs=NKT) as vpool,
        ):
            # ---- constants / tables ----
            onesb = constp.tile([128, 64], BF16, tag="onesb")
            nc.vector.memset(onesb[:], 1.0)
            eps_sb = constp.tile([128, 1], F32, tag="eps")
            nc.vector.memset(eps_sb[:], EPS)
            wq_sb = constp.tile([128, NDT * C], BF16, tag="wq")
            c4q_sb = constp.tile([128, NQ], BF16, tag="c4q")
            s4q_sb = constp.tile([128, NQ], BF16, tag="s4q")
            perm_sb = constp.tile([128, 128], BF16, tag="perm")
            idw_sb = constp.tile([128, 128], BF16, tag="idw")
            qso_sb = constp.tile([128, 131], BF16, tag="qso")
            xq_sb = constp.tile([128, 2 * NDT * 512], BF16, tag="xq")
            wk_sb = constp.tile([128, NDT * C], BF16, tag="wk")
            wv_sb = constp.tile([128, NDT * C], BF16, tag="wv")
            c4k_sb = constp.tile([128, T], BF16, tag="c4k")
            s4k_sb = constp.tile([128, T], BF16, tag="s4k")
            m01_sb = constp.tile([128, NKT * 128], BF16, tag="m01")
            wpp_sb = constp.tile([64, 3 * D], BF16, tag="wpp")

            # Q-path inputs first (sync ring); small consts on the
            # gpsimd ring so they don't head-of-line block the big ones
            nc.sync.dma_start(wq_sb[:], wq[:, :])
            for ch in range(2):
                nc.sync.dma_start(
                    xq_sb[:, 3072 * ch : 3072 * (ch + 1)],
                    xqp[:, 3072 * ch : 3072 * (ch + 1)],
                )
            nc.gpsimd.dma_start(perm_sb[:], perm[:, :])
            nc.gpsimd.dma_start(idw_sb[:], idw[:, :])
            nc.gpsimd.dma_start(qso_sb[:], qso[:, :])
            nc.gpsimd.dma_start(c4q_sb[:], c4q[:, :])
            nc.gpsimd.dma_start(s4q_sb[:], s4q[:, :])
            nc.sync.dma_start(wk_sb[:], wk[:, :])
            nc.sync.dma_start(wv_sb[:], wv[:, :])
            nc.gpsimd.dma_start(c4k_sb[:], c4k[:, :])
            nc.gpsimd.dma_start(s4k_sb[:], s4k[:, :])

            # ---- persistent q/k tiles (A: heads 0,1  B: head 2) ----
            qA = pers.tile([128, NQ], BF16, tag="qA")
            qB = pers.tile([64, NQ], BF16, tag="qB")
            kA = pers.tile([128, T], BF16, tag="kA")
            kB = pers.tile([64, T], BF16, tag="kB")
            rk_bf = pers.tile([65, T], BF16, tag="rkbf")
            rk2_bf = pers.tile([1, T], BF16, tag="rk2bf")
            rq_bf = pers.tile([65, NQ], BF16, tag="rqbf")
            rq2_bf = pers.tile([1, NQ], BF16, tag="rq2bf")

            v_sb = []
            for t in range(NKT):
                vt = vpool.tile([128, 3 * 128], BF16, tag="v", name=f"v{t}")
                nc.vector.memset(vt[:], 0.0)
                v_sb.append(vt)

            with (
                tc.tile_pool(name="xtp", bufs=2) as xtpool,
                tc.tile_pool(name="vinp", bufs=2) as vinpool,
                tc.tile_pool(name="sqp", bufs=2) as sqp,
                tc.tile_pool(name="ropep", bufs=4) as ropep,
                tc.tile_pool(name="rowp", bufs=2) as rowp,
                tc.tile_pool(name="psA", bufs=2, space="PSUM") as psA,
                tc.tile_pool(name="psB", bufs=1, space="PSUM") as psB,
                tc.tile_pool(name="psrb", bufs=2, space="PSUM") as psrb,
                tc.tile_pool(name="psq", bufs=2, space="PSUM") as psqp,
            ):
                # ---- Q projection + per-chunk rms rows (x^-0.5 via
                # Exp(-0.5 Ln(x)) on ACT; h0/h2 rows at psum 0/64, h1 its own)
                for ch in range(2):
                    c0 = 512 * ch
                    sq_a = sqp.tile([128, 512], BF16, tag="sq", name="sqa")
                    sq_b = sqp.tile([64, 512], BF16, tag="sq", name="sqb")
                    for dst, m, coff, sqt in (
                        (qA, 128, 0, sq_a),
                        (qB, 64, 128, sq_b),
                    ):
                        ps = psA.tile([m, 512], F32, tag="psA", name="psq")
                        for d in range(NDT):
                            nc.tensor.matmul(
                                ps[:],
                                wq_sb[:, C * d + coff : C * d + coff + m],
                                xq_sb[:, 3072 * ch + 512 * d : 3072 * ch + 512 * d + 512],
                                start=(d == 0),
                                stop=(d == NDT - 1),
                            )
                        nc.vector.tensor_copy(dst[:, c0 : c0 + 512], ps[:])
                        nc.scalar.activation(sqt[:], ps[0 : sqt.shape[0], :], AF.Square)
                    if "qssq" in SKIP:
                        nc.vector.memset(rq_bf[:, c0 : c0 + 512], 1.0)
                        nc.vector.memset(rq2_bf[:, c0 : c0 + 512], 1.0)
                        continue
                    qssq1 = psqp.tile([65, 512], F32, tag="qssq", name="qssq1")
                    qssq2 = psqp.tile([1, 512], F32, tag="qssq", name="qssq2")
                    nc.tensor.matmul(
                        qssq1[:], qso_sb[:, 0:65], sq_a[:],
                        start=True, stop=False, skip_group_check=True,
                    )
                    nc.tensor.matmul(
                        qssq1[:], qso_sb[0:64, 65:130], sq_b[:],
                        start=False, stop=True, skip_group_check=True,
                    )
                    nc.tensor.matmul(
                        qssq2[:], qso_sb[:, 130:131], sq_a[:],
                        start=True, stop=True, skip_group_check=True,
                    )
                    ln1 = rowp.tile([65, 512], F32, tag="rqs", name="ln1")
                    ln2 = rowp.tile([1, 512], F32, tag="rqs", name="ln2")
                    nc.scalar.activation(
                        ln1[:], qssq1[:], AF.Ln, bias=eps_sb[0:65, :], scale=1.0 / HD
                    )
                    nc.scalar.activation(
                        ln2[:], qssq2[:], AF.Ln, bias=eps_sb[0:1, :], scale=1.0 / HD
                    )
                    nc.scalar.activation(
                        rq_bf[:, c0 : c0 + 512], ln1[:], AF.Exp, scale=-0.5
                    )
                    nc.scalar.activation(
                        rq2_bf[:, c0 : c0 + 512], ln2[:], AF.Exp, scale=-0.5
                    )

                # ---- Q rope + rms apply, in place ----
                for ch in range([2, 0]["qrope" in SKIP]):
                    c0 = 512 * ch
                    rb = psrb.tile([128, 512], F32, tag="rb", name="rb")
                    rbB = psrb.tile([64, 512], F32, tag="rb", name="rbB")
                    nc.tensor.matmul(
                        rb[0:64, :],
                        onesb[0:1, 0:64],
                        rq_bf[0:1, c0 : c0 + 512],
                        start=True, stop=True,
                        tile_position=(0, 0), skip_group_check=True,
                    )
                    nc.tensor.matmul(
                        rb[64:128, :],
                        onesb[0:1, 0:64],
                        rq2_bf[0:1, c0 : c0 + 512],
                        start=True, stop=True,
                        tile_position=(0, 64), skip_group_check=True,
                    )
                    nc.tensor.matmul(
                        rbB[:],
                        onesb[64:65, 0:64],
                        rq_bf[64:65, c0 : c0 + 512],
                        start=True, stop=True,
                        tile_position=(64, 0), skip_group_check=True,
                    )
                    rb_sb = ropep.tile([128, 512], F32, tag="ropef", name="rbsb")
                    nc.vector.tensor_copy(rb_sb[:], rb[:])
                    rbB_sb = ropep.tile([64, 512], F32, tag="ropef", name="rbBsb")
                    nc.vector.tensor_copy(rbB_sb[:], rbB[:])
                    for tile_, P, rbt in ((qA, 128, rb_sb), (qB, 64, rbB_sb)):
                        sl = tile_[:, c0 : c0 + 512]
                        m_ = ropep.tile([128, 512], BF16, tag="rope", name="m_")
                        e1 = ropep.tile([128, 512], BF16, tag="rope", name="e1")
                        nc.vector.tensor_mul(m_[0:P, :], sl, c4q_sb[0:P, c0 : c0 + 512])
                        nc.vector.tensor_mul(e1[0:P, :], sl, s4q_sb[0:P, c0 : c0 + 512])
                        qs = psB.tile([128, 512], F32, tag="qs", name="qs")
                        nc.tensor.matmul(
                            qs[0:P, :],
                            perm_sb[0:P, 0:P],
                            e1[0:P, :],
                            start=True, stop=False, skip_group_check=True,
                        )
                        nc.tensor.matmul(
                            qs[0:P, :],
                            idw_sb[0:P, 0:P],
                            m_[0:P, :],
                            start=False, stop=True, skip_group_check=True,
                        )
                        nc.vector.tensor_mul(sl, qs[0:P, :], rbt[0:P, :])

                # ---- K + V per 512-col chunk; kcol ssq per key tile ----
                for ch in range(4):
                    c0 = 512 * ch
                    xt_ch = xtpool.tile([128, 3072], BF16, tag="xt", name="xt")
                    nc.sync.dma_start(
                        xt_ch[:], xtp[:, 3072 * ch : 3072 * (ch + 1)]
                    )
                    vi_t = vinpool.tile([128, 4 * C], F32, tag="vin")
                    nc.sync.dma_start(
                        vi_t[:], vin[:, 4 * C * ch : 4 * C * (ch + 1)]
                    )
                    sq_a = sqp.tile([128, 512], BF16, tag="sq", name="ksqa")
                    sq_b = sqp.tile([64, 512], BF16, tag="sq", name="ksqb")
                    for dst, m, coff, sqt in (
                        (kA, 128, 0, sq_a),
                        (kB, 64, 128, sq_b),
                    ):
                        ps = psA.tile([m, 512], F32, tag="psA", name="psk")
                        for d in range(NDT):
                            nc.tensor.matmul(
                                ps[:],
                                wk_sb[:, C * d + coff : C * d + coff + m],
                                xt_ch[:, 512 * d : 512 * d + 512],
                                start=(d == 0),
                                stop=(d == NDT - 1),
                            )
                        nc.vector.tensor_copy(dst[:, c0 : c0 + 512], ps[:])
                        nc.scalar.activation(sqt[:], ps[0 : sqt.shape[0], :], AF.Square)
                    # row-form ssq + x^-0.5, mirroring the Q path
                    kssq1 = psqp.tile([65, 512], F32, tag="qssq", name="kssq1")
                    kssq2 = psqp.tile([1, 512], F32, tag="qssq", name="kssq2")
                    nc.tensor.matmul(
                        kssq1[:], qso_sb[:, 0:65], sq_a[:],
                        start=True, stop=False, skip_group_check=True,
                    )
                    nc.tensor.matmul(
                        kssq1[:], qso_sb[0:64, 65:130], sq_b[:],
                        start=False, stop=True, skip_group_check=True,
                    )
                    nc.tensor.matmul(
                        kssq2[:], qso_sb[:, 130:131], sq_a[:],
                        start=True, stop=True, skip_group_check=True,
                    )
                    kln1 = rowp.tile([65, 512], F32, tag="rqs", name="kln1")
                    kln2 = rowp.tile([1, 512], F32, tag="rqs", name="kln2")
                    nc.scalar.activation(
                        kln1[:], kssq1[:], AF.Ln, bias=eps_sb[0:65, :], scale=1.0 / HD
                    )
                    nc.scalar.activation(
                        kln2[:], kssq2[:], AF.Ln, bias=eps_sb[0:1, :], scale=1.0 / HD
                    )
                    nc.scalar.activation(
                        rk_bf[:, c0 : c0 + 512], kln1[:], AF.Exp, scale=-0.5
                    )
                    nc.scalar.activation(
                        rk2_bf[:, c0 : c0 + 512], kln2[:], AF.Exp, scale=-0.5
                    )
                    # rb broadcast for k (proven Q pattern), then rope with
                    # the rms scale folded into the final evac multiply
                    rbk = psrb.tile([128, 512], F32, tag="rb", name="rbk")
                    rbkB = psrb.tile([64, 512], F32, tag="rb", name="rbkB")
                    nc.tensor.matmul(
                        rbk[0:64, :], onesb[0:1, 0:64], rk_bf[0:1, c0 : c0 + 512],
                        start=True, stop=True, tile_position=(0, 0),
                        skip_group_check=True,
                    )
                    nc.tensor.matmul(
                        rbk[64:128, :], onesb[0:1, 0:64], rk2_bf[0:1, c0 : c0 + 512],
                        start=True, stop=True, tile_position=(0, 64),
                        skip_group_check=True,
                    )
                    nc.tensor.matmul(
                        rbkB[:], onesb[64:65, 0:64], rk_bf[64:65, c0 : c0 + 512],
                        start=True, stop=True, tile_position=(64, 0),
                        skip_group_check=True,
                    )
                    rbk_sb = ropep.tile([128, 512], F32, tag="ropef", name="rbksb")
                    nc.vector.tensor_copy(rbk_sb[:], rbk[:])
                    rbkB_sb = ropep.tile([64, 512], F32, tag="ropef", name="rbkBsb")
                    nc.vector.tensor_copy(rbkB_sb[:], rbkB[:])
                    for tile_, P, rbt in [
                        ((kA, 128, rbk_sb), (kB, 64, rbkB_sb)),
                        (),
                    ]["krope" in SKIP]:
                        sl = tile_[:, c0 : c0 + 512]
                        m_ = ropep.tile([128, 512], BF16, tag="rope", name="km_")
                        e1 = ropep.tile([128, 512], BF16, tag="rope", name="ke1")
                        nc.vector.tensor_mul(m_[0:P, :], sl, c4k_sb[0:P, c0 : c0 + 512])
                        nc.vector.tensor_mul(e1[0:P, :], sl, s4k_sb[0:P, c0 : c0 + 512])
                        qs = psB.tile([128, 512], F32, tag="qs", name="kqs")
                        nc.tensor.matmul(
                            qs[0:P, :],
                            perm_sb[0:P, 0:P],
                            e1[0:P, :],
                            start=True,
                            stop=False,
                            skip_group_check=True,
                        )
                        nc.tensor.matmul(
                            qs[0:P, :],
                            idw_sb[0:P, 0:P],
                            m_[0:P, :],
                            start=False,
                            stop=True,
                            skip_group_check=True,
                        )
                        nc.vector.tensor_mul(sl, qs[0:P, :], rbt[0:P, :])
                    # V for the 4 key tiles inside this chunk (natural layout)
                    for ti in range([4, 0]["vproj" in SKIP]):
                        t = 4 * ch + ti
                        ps = psA.tile([128, 512], F32, tag="psA", name="psv")
                        for d in range(NDT):
                            nc.tensor.matmul(
                                ps[:, 0:C],
                                xt_ch[:, 512 * d + 128 * ti : 512 * d + 128 * ti + 128],
                                wv_sb[:, C * d : C * (d + 1)],
                                start=(d == 0),
                                stop=(d == NDT - 1),
                            )
                        vt = v_sb[t]
                        dst3 = vt[:].rearrange("p (h c) -> p h c", h=3, c=128)[:, :, 0:64]
                        src3 = ps[:, 0:C].rearrange("p (h c) -> p h c", h=3)
                        vin3 = vi_t[:, C * ti : C * (ti + 1)].rearrange(
                            "p (h c) -> p h c", h=3
                        )
                        nc.vector.tensor_add(dst3, src3, vin3)
                        nc.vector.memset(
                            vt[:].rearrange("p (h c) -> p h c", h=3, c=128)[:, :, 64:65],
                            1.0,
                        )

            nc.gpsimd.dma_start(m01_sb[:], m01[:, :])
            nc.gpsimd.dma_start(wpp_sb[:], wpp[:, :])

            # ---- attention ----
            if KPHASE < 2:
                with tc.tile_pool(name="yz", bufs=1) as yzp:
                    zt = yzp.tile([128, D], F32, tag="z")
                    nc.vector.memset(zt[:], 0.0)
                    for j in range(8):
                        nc.sync.dma_start(yp[128 * j : 128 * (j + 1), :], zt[:])
                return nc

            with (
                tc.tile_pool(name="atp", bufs=1) as atp,
                tc.tile_pool(name="epool", bufs=4) as epool,
                tc.tile_pool(name="rowd", bufs=2) as rowd,
            ):
              with (
                tc.tile_pool(name="pvp", bufs=2, space="PSUM") as pvp,
                tc.tile_pool(name="stp", bufs=4, space="PSUM") as stp,
              ):
                at_sb = [
                    atp.tile([64, NQ], BF16, tag=f"at{h}", name=f"at{h}")
                    for h in range(3)
                ]

                def norm_recip(pv):
                    rden = rowd.tile([1, NQ], F32, tag="rden", name="rden")
                    nc.vector.reciprocal(rden[:], pv[64:65, :])
                    rden_bf = rowd.tile([1, NQ], BF16, tag="rden", name="rdenbf")
                    nc.vector.tensor_copy(rden_bf[:], rden[:])
                    return rden_bf

                def norm_recip_act(pv):
                    # ACT-side reciprocal for the tail head: exp(-ln(x))
                    lnd = rowd.tile([1, NQ], F32, tag="rden", name="lnd")
                    nc.scalar.activation(lnd[:], pv[64:65, :], AF.Ln)
                    rden_bf = rowd.tile([1, NQ], BF16, tag="rden", name="rdenbf")
                    nc.scalar.activation(rden_bf[:], lnd[:], AF.Exp, scale=-1.0)
                    return rden_bf

                def norm_apply(h, pv, rden_bf, c0s):
                    for c0 in c0s:
                        rbn = stp.tile([64, 512], F32, tag="st", name="rbn")
                        nc.tensor.matmul(
                            rbn[:],
                            onesb[0:1, 0:64],
                            rden_bf[:, c0 : c0 + 512],
                            start=True,
                            stop=True,
                            skip_group_check=True,
                        )
                        rbn_sb = rowd.tile([64, 512], F32, tag="rbns", name="rbns")
                        nc.vector.tensor_copy(rbn_sb[:], rbn[:])
                        nc.vector.tensor_mul(
                            at_sb[h][:, c0 : c0 + 512],
                            pv[0:64, c0 : c0 + 512],
                            rbn_sb[:],
                        )

                pvs = {}
                rdens = {}
                for h in range(3):
                    kr = kA if h < 2 else kB
                    qr = qA if h < 2 else qB
                    poff = 64 * (h % 2)
                    pv = pvp.tile([128, NQ], F32, tag="pv", name="pv")
                    pvs[h] = pv
                    for k in range(NKT):
                        if k == 2 and h >= 1:
                            rdens[h - 1] = norm_recip(pvs[h - 1])
                        if k == 12 and h >= 1:
                            norm_apply(h - 1, pvs[h - 1], rdens[h - 1], (0, 512))
                        q0 = 128 * j0_of(k)
                        for pi, c0 in enumerate(
                            range(q0 - (q0 % 512), NQ, 512)
                        ):
                            a0 = max(c0, q0)
                            st = stp.tile([128, 512], F32, tag="st", name="st")
                            nc.tensor.matmul(
                                st[:, 0 : c0 + 512 - a0],
                                kr[poff : poff + 64, 128 * k : 128 * (k + 1)],
                                qr[poff : poff + 64, a0 : c0 + 512],
                                start=True,
                                stop=True,
                                skip_group_check=True,
                            )
                            et = epool.tile([128, 512], BF16, tag="e", name="et")
                            nc.scalar.activation(
                                et[:, 0 : c0 + 512 - a0],
                                st[:, 0 : c0 + 512 - a0],
                                AF.Exp,
                            )
                            if pi == 0:
                                nc.gpsimd.tensor_mul(
                                    et[:, 0:128],
                                    et[:, 0:128],
                                    m01_sb[:, 128 * k : 128 * (k + 1)],
                                )
                            nc.tensor.matmul(
                                pv[:, a0 : c0 + 512],
                                v_sb[k][:, 128 * h : 128 * h + 128],
                                et[:, 0 : c0 + 512 - a0],
                                start=(k == 0),
                                stop=(k == NKT - 1),
                                skip_group_check=True,
                            )

                rd2 = norm_recip_act(pvs[2])
                norm_apply(2, pvs[2], rd2, (0, 512))

              # ---- output projection ----
              if KPHASE < 3:
                  with tc.tile_pool(name="yz2", bufs=1) as yzp:
                      zt = yzp.tile([128, D], F32, tag="z")
                      nc.vector.memset(zt[:], 0.0)
                      for j in range(8):
                          nc.sync.dma_start(yp[128 * j : 128 * (j + 1), :], zt[:])
                  return nc
              with (
                  tc.tile_pool(name="psy", bufs=2, space="PSUM") as psyp,
                  tc.tile_pool(name="ypool", bufs=2) as ypool,
              ):
                  for j in range(8):
                      ps = psyp.tile([128, D], F32, tag="psy", name="psy")
                      for n0, n1 in ((0, 512), (512, D)):
                          for h in range(3):
                              nc.tensor.matmul(
                                  ps[:, n0:n1],
                                  at_sb[h][:, 128 * j : 128 * (j + 1)],
                                  wpp_sb[:, D * h + n0 : D * h + n1],
                                  start=(h == 0),
                                  stop=(h == 2),
                                  skip_group_check=True,
                              )
                      yt = ypool.tile([128, D], F32, tag="y")
                      nc.vector.tensor_copy(yt[:], ps[:])
                      nc.sync.dma_start(yp[128 * j : 128 * (j + 1), :], yt[:])

    return nc


def _host_prep(x, vi, Wq, Wk, Wv, Wp, lamb):
    import ml_dtypes

    BF = ml_dtypes.bfloat16
    lam = float(lamb)
    xtf = np.ascontiguousarray(x[0].T, dtype=np.float32)  # [768, 2048]

    inv_freq = (1.0 / 10000.0) ** (np.arange(0, HD, 2, dtype=np.float32) / HD)
    tpos = np.arange(T, dtype=np.float32)
    freqs = np.outer(tpos, inv_freq).astype(np.float32)  # [T, 32]
    cosT = np.cos(freqs).T.astype(np.float32)  # [32, T]
    sinT = np.sin(freqs).T.astype(np.float32)
    c4 = np.vstack([cosT] * 4)  # [128, T]
    # swapped-sign sin stack: t_ = perm(raw * s4sw); perm swaps 0:32<->32:64
    #   rows 0:32 hold -sin (land on y2 = x2 c - x1 s)
    #   rows 32:64 hold +sin (land on y1 = x1 c + x2 s)
    s4sw = np.vstack([-sinT, sinT, -sinT, sinT])
    scale = float(1.0 / np.sqrt(np.float32(HD)))

    permf = np.zeros((128, 128), dtype=np.float32)
    for b in range(2):
        for i in range(32):
            permf[64 * b + 32 + i, 64 * b + i] = 1.0
            permf[64 * b + i, 64 * b + 32 + i] = 1.0
    idf = np.eye(128, dtype=np.float32)

    qsof = np.zeros((128, 131), dtype=np.float32)
    qsof[0:64, 0] = 1.0  # A block: head 0 -> qssq1 row 0
    qsof[0:64, 65 + 64] = 1.0  # B block: head 2 -> qssq1 row 64
    qsof[64:128, 130] = 1.0  # A2 sel: head 1 -> qssq2 row 0

    tri01 = (np.arange(128)[None, :] >= np.arange(128)[:, None]).astype(
        np.float32
    )  # [p=tk, c=tq]: allowed iff c >= p

    qcols_by_par = {}
    for par in (0, 1):
        jj = np.arange(8)
        qcols_by_par[par] = (
            256 * jj[:, None] + 128 * par + np.arange(128)[None, :]
        ).reshape(-1)

    def pack_chunks(mat, n_ch):
        # [768, n_ch*512] -> [128, n_ch*6*512] chunk-major, d-blocks inside
        cols = []
        for ch in range(n_ch):
            for d in range(NDT):
                cols.append(mat[128 * d : 128 * (d + 1), 512 * ch : 512 * (ch + 1)])
        return np.ascontiguousarray(np.concatenate(cols, axis=1))

    def pack_w(w):  # [768, 192] -> [128, 6*192]
        return np.ascontiguousarray(
            np.concatenate(
                [w[128 * d : 128 * (d + 1), :] for d in range(NDT)], axis=1
            )
        )

    in_maps = []
    for core in range(8):
        g, par = core // 2, core % 2
        cg = slice(C * g, C * (g + 1))
        qcols = qcols_by_par[par]
        mask = np.zeros((128, NKT * 128), dtype=np.float32)
        for k in range(NKT):
            gtile = 2 * j0_of(k) + par
            if gtile == k:
                mask[:, 128 * k : 128 * (k + 1)] = tri01
            elif gtile > k:
                mask[:, 128 * k : 128 * (k + 1)] = 1.0
        vinm = lam * vi[0][:, cg]  # [2048, 192]
        vinp = np.concatenate(
            [vinm[128 * t : 128 * (t + 1), :] for t in range(NKT)], axis=1
        )
        wpt = Wp[:, cg].T  # [192, 768]
        in_maps.append(
            {
                "xtp": pack_chunks(xtf, 4).astype(BF),
                "xqp": pack_chunks(
                    np.ascontiguousarray(xtf[:, qcols]), 2
                ).astype(BF),
                "wq": pack_w(Wq[cg, :].T).astype(BF),
                "wk": pack_w(Wk[cg, :].T).astype(BF),
                "wv": pack_w((1.0 - lam) * Wv[cg, :].T).astype(BF),
                "wpp": np.ascontiguousarray(
                    np.concatenate(
                        [wpt[64 * h : 64 * (h + 1), :] for h in range(3)],
                        axis=1,
                    )
                ).astype(BF),
                "vin": np.ascontiguousarray(vinp, dtype=np.float32),
                "c4k": c4.astype(BF),
                "s4k": s4sw.astype(BF),
                "c4q": np.ascontiguousarray(scale * c4[:, qcols]).astype(BF),
                "s4q": np.ascontiguousarray(scale * s4sw[:, qcols]).astype(BF),
                "m01": mask.astype(BF),
                "perm": permf.astype(BF),
                "idw": idf.astype(BF),
                "qso": qsof.astype(BF),
            }
        )
    return in_maps, qcols_by_par


def kernel(x, vi, Wq, Wk, Wv, Wp, lamb):
    from concourse.bass_utils import run_bass_kernel_spmd

    x = np.asarray(x, dtype=np.float32)
    vi = np.asarray(vi, dtype=np.float32)
    Wq = np.asarray(Wq, dtype=np.float32)
    Wk = np.asarray(Wk, dtype=np.float32)
    Wv = np.asarray(Wv, dtype=np.float32)
    Wp = np.asarray(Wp, dtype=np.float32)

    in_maps, qcols_by_par = _host_prep(x, vi, Wq, Wk, Wv, Wp, lamb)
    if "nc" not in _CACHED:
        _CACHED["nc"] = build_nc()
    nc = _CACHED["nc"]
    res = run_bass_kernel_spmd(
        nc, in_maps, core_ids=list(range(8)), trace=TRACE, tmpdir=TRACE_DIR
    )
    _CACHED["last_result"] = res

    y = np.zeros((T, D), dtype=np.float32)
    for core in range(8):
        y[qcols_by_par[core % 2]] += res.results[core]["yp"]
    return y[None]

